# revision 1
# baseline (speedup 1.0000x reference)
import sys

if "/opt/trn_rl_repo" not in sys.path:
    sys.path.insert(0, "/opt/trn_rl_repo")

import numpy as np

import concourse.mybir as mybir
import concourse.tile as tile
from concourse import bacc, bass_utils
from concourse.masks import make_identity

N_CORES = 8
B, IN, H, OUT = 16384, 784, 4096, 10
BN_EPS = 1e-5

f32 = mybir.dt.float32
bf16 = mybir.dt.bfloat16
f16 = mybir.dt.float16
AF = mybir.ActivationFunctionType
ALU = mybir.AluOpType


def build_nc(b_sh=B // N_CORES, h_dim=H, n_cores=N_CORES, use_collective=True):
    inp = 896
    nk = inp // 128
    nm = h_dim // 128
    nbt = b_sh // 128
    nch = b_sh // 512
    batch_total = b_sh * n_cores

    nc = bacc.Bacc("TRN2", target_bir_lowering=False, debug=False,
                   num_devices=n_cores)

    x_in = nc.dram_tensor("x", [b_sh, IN], f32, kind="ExternalInput").ap()
    w1_in = nc.dram_tensor("W1", [h_dim, IN], f32, kind="ExternalInput").ap()
    gamma_in = nc.dram_tensor("gamma", [h_dim], f32, kind="ExternalInput").ap()
    beta_in = nc.dram_tensor("beta", [h_dim], f32, kind="ExternalInput").ap()
    w2_in = nc.dram_tensor("W2", [OUT, h_dim], f32, kind="ExternalInput").ap()
    out_d = nc.dram_tensor("out", [b_sh, OUT], f32, kind="ExternalOutput").ap()

    with tile.TileContext(nc) as tc:
        with (
            tc.tile_pool(name="const", bufs=1) as const,
            tc.tile_pool(name="dram", bufs=1, space="DRAM") as dram,
        ):
            ident = const.tile([128, 128], f32)
            make_identity(nc, ident[:])
            sW2T = const.tile([128, nm, OUT], bf16)
            gamma_pm = const.tile([128, nm], f32)
            beta_pm = const.tile([128, nm], f32)
            scale_pm = const.tile([128, nm], f32)
            bias_pm = const.tile([128, nm], f32)
            stats = const.tile([128, 2 * nm], f32)
            gstats = const.tile([128, 2 * nm], f32)

            xhi_d = dram.tile([b_sh, inp], f16)
            xlo_d = dram.tile([b_sh, inp], bf16)
            sw1_d = dram.tile([h_dim, inp], bf16)
            h_d = dram.tile([h_dim, b_sh], f32)
            cc_in = dram.tile([128, 2 * nm], f32)
            cc_out = dram.tile([128, 2 * nm], f32)

            with tc.tile_pool(name="persist", bufs=1) as persist:
                xhiT = [persist.tile([128, b_sh], f16, name=f"xhiT{k}")
                        for k in range(nk)]
                xloT = [persist.tile([128, b_sh], bf16, name=f"xloT{k}")
                        for k in range(nk)]
                sW1T = [persist.tile([128, h_dim], bf16, name=f"sW1T{k}")
                        for k in range(nk)]

                with (
                    tc.tile_pool(name="prolog", bufs=3) as prolog,
                    tc.tile_pool(name="pps", bufs=2, space="PSUM") as pps,
                ):
                    w2_sb = prolog.tile([OUT, h_dim], f32, tag="w2sb")
                    nc.sync.dma_start(w2_sb[:], w2_in)
                    for m in range(nm):
                        pt = pps.tile([128, OUT], f32, tag="w2t")
                        nc.tensor.transpose(
                            pt[:], w2_sb[:OUT, m * 128:(m + 1) * 128],
                            ident[:OUT, :OUT])
                        nc.scalar.activation(sW2T[:, m, :], pt[:], AF.Sign)

                    ga_sb = prolog.tile([nm, 128], f32, tag="gasb")
                    be_sb = prolog.tile([nm, 128], f32, tag="besb")
                    nc.sync.dma_start(
                        ga_sb[:], gamma_in.rearrange("(m p) -> m p", p=128))
                    nc.sync.dma_start(
                        be_sb[:], beta_in.rearrange("(m p) -> m p", p=128))
                    ga_ps = pps.tile([128, nm], f32, tag="gaps")
                    nc.tensor.transpose(ga_ps[:], ga_sb[:], ident[:nm, :nm])
                    nc.scalar.copy(gamma_pm[:], ga_ps[:])
                    be_ps = pps.tile([128, nm], f32, tag="beps")
                    nc.tensor.transpose(be_ps[:], be_sb[:], ident[:nm, :nm])
                    nc.scalar.copy(beta_pm[:], be_ps[:])

                    for t in range(nbt):
                        xt = prolog.tile([128, IN], f32, tag="xt")
                        nc.sync.dma_start(
                            xt[:], x_in[t * 128:(t + 1) * 128, :])
                        xhi = prolog.tile([128, inp], f16, tag="xhi")
                        xlo = prolog.tile([128, inp], bf16, tag="xlo")
                        nc.vector.memset(xhi[:, IN:], 0.0)
                        nc.vector.memset(xlo[:, IN:], 0.0)
                        nc.vector.tensor_copy(xhi[:, :IN], xt[:])
                        nc.vector.tensor_tensor(
                            xlo[:, :IN], xt[:], xhi[:, :IN], op=ALU.subtract)
                        nc.sync.dma_start(
                            xhi_d[t * 128:(t + 1) * 128, :], xhi[:])
                        nc.sync.dma_start(
                            xlo_d[t * 128:(t + 1) * 128, :], xlo[:])

                    for m in range(nm):
                        w1t = prolog.tile([128, IN], f32, tag="w1t")
                        nc.sync.dma_start(
                            w1t[:], w1_in[m * 128:(m + 1) * 128, :])
                        sw1 = prolog.tile([128, inp], bf16, tag="sw1")
                        nc.vector.memset(sw1[:, IN:], 0.0)
                        nc.scalar.activation(sw1[:, :IN], w1t[:], AF.Sign)
                        nc.sync.dma_start(
                            sw1_d[m * 128:(m + 1) * 128, :], sw1[:])

                    for k in range(nk):
                        nc.sync.dma_start_transpose(
                            xhiT[k][:], xhi_d[:, k * 128:(k + 1) * 128])
                        nc.sync.dma_start_transpose(
                            xloT[k][:], xlo_d[:, k * 128:(k + 1) * 128])
                        nc.sync.dma_start_transpose(
                            sW1T[k][:], sw1_d[:, k * 128:(k + 1) * 128])

                with (
                    tc.tile_pool(name="ph1", bufs=3) as ph1,
                    tc.tile_pool(name="ph1sq", bufs=2) as ph1sq,
                    tc.tile_pool(name="ps1", bufs=2, space="PSUM") as ps1,
                ):
                    for m in range(nm):
                        ph_t = ps1.tile([128, b_sh], f32, tag="ph")
                        for li, xT in enumerate((xhiT, xloT)):
                            for k in range(nk):
                                lhsT = sW1T[k][:, m * 128:(m + 1) * 128]
                                for c in range(nch):
                                    nc.tensor.matmul(
                                        ph_t[:, c * 512:(c + 1) * 512],
                                        lhsT,
                                        xT[k][:, c * 512:(c + 1) * 512],
                                        start=(li == 0 and k == 0),
                                        stop=(li == 1 and k == nk - 1),
                                    )
                        h_sb = ph1.tile([128, b_sh], f32, tag="hsb")
                        nc.scalar.activation(
                            h_sb[:], ph_t[:], AF.Identity,
                            accum_out=stats[:, m:m + 1])
                        sq_sb = ph1sq.tile([128, b_sh], bf16, tag="sqsb")
                        nc.scalar.activation(
                            sq_sb[:], ph_t[:], AF.Square,
                            accum_out=stats[:, nm + m:nm + m + 1])
                        nc.sync.dma_start(
                            h_d[m * 128:(m + 1) * 128, :], h_sb[:])

            nc.sync.dma_start(cc_in[:], stats[:])
            if use_collective:
                nc.gpsimd.collective_compute(
                    "AllReduce", ALU.add,
                    replica_groups=[list(range(n_cores))],
                    ins=[cc_in.opt()], outs=[cc_out.opt()],
                )
            else:
                nc.sync.dma_start(cc_out[:], cc_in[:])
            nc.sync.dma_start(gstats[:], cc_out[:])

            mean_t = const.tile([128, nm], f32)
            var_t = const.tile([128, nm], f32)
            tmp_t = const.tile([128, nm], f32)
            nc.vector.tensor_scalar_mul(
                mean_t[:], gstats[:, :nm], 1.0 / batch_total)
            nc.vector.tensor_scalar_mul(
                var_t[:], gstats[:, nm:], 1.0 / batch_total)
            nc.vector.tensor_tensor(tmp_t[:], mean_t[:], mean_t[:], op=ALU.mult)
            nc.vector.tensor_tensor(var_t[:], var_t[:], tmp_t[:], op=ALU.subtract)
            nc.vector.tensor_scalar_add(var_t[:], var_t[:], BN_EPS)
            nc.vector.reciprocal(tmp_t[:], var_t[:])
            nc.scalar.activation(tmp_t[:], tmp_t[:], AF.Sqrt)
            nc.vector.tensor_tensor(
                scale_pm[:], tmp_t[:], gamma_pm[:], op=ALU.mult)
            nc.vector.tensor_tensor(tmp_t[:], mean_t[:], scale_pm[:], op=ALU.mult)
            nc.vector.tensor_tensor(
                bias_pm[:], beta_pm[:], tmp_t[:], op=ALU.subtract)

            with (
                tc.tile_pool(name="ph2", bufs=3) as ph2,
                tc.tile_pool(name="ph2s", bufs=2) as ph2s,
                tc.tile_pool(name="ep", bufs=1) as ep,
                tc.tile_pool(name="ps2", bufs=1, space="PSUM") as ps2,
            ):
                psL = ps2.tile([OUT, b_sh], f32)
                for m in range(nm):
                    h2 = ph2.tile([128, b_sh], f32, tag="h2")
                    nc.sync.dma_start(h2[:], h_d[m * 128:(m + 1) * 128, :])
                    s_t = ph2s.tile([128, b_sh], bf16, tag="st")
                    nc.scalar.activation(
                        s_t[:], h2[:], AF.Sign,
                        bias=bias_pm[:, m:m + 1], scale=scale_pm[:, m:m + 1])
                    for c in range(nch):
                        nc.tensor.matmul(
                            psL[:, c * 512:(c + 1) * 512],
                            sW2T[:, m:m + 1, :],
                            s_t[:, c * 512:(c + 1) * 512],
                            start=(m == 0), stop=(m == nm - 1),
                        )

                LT = ep.tile([OUT, b_sh], f32)
                nc.scalar.copy(LT[:], psL[:])
                psT = ps2.tile([128, nbt * OUT], f32)
                for t in range(nbt):
                    nc.tensor.transpose(
                        psT[:, t * OUT:(t + 1) * OUT],
                        LT[:OUT, t * 128:(t + 1) * 128],
                        ident[:OUT, :OUT])
                Lb = ep.tile([128, nbt, OUT], f32)
                nc.scalar.copy(Lb[:], psT[:])

                negmax = ep.tile([128, nbt], f32)
                nc.vector.tensor_reduce(
                    negmax[:], Lb[:], axis=mybir.AxisListType.X,
                    op=ALU.max, negate=True)
                shifted = ep.tile([128, nbt, OUT], f32)
                nc.vector.tensor_tensor(
                    shifted[:], Lb[:],
                    negmax[:][:, :, None].broadcast_to([128, nbt, OUT]),
                    op=ALU.add)
                expv = ep.tile([128, nbt, OUT], f32)
                nc.scalar.activation(expv[:], shifted[:], AF.Exp)
                sumexp = ep.tile([128, nbt], f32)
                nc.vector.tensor_reduce(
                    sumexp[:], expv[:], axis=mybir.AxisListType.X, op=ALU.add)
                lse = ep.tile([128, nbt], f32)
                nc.scalar.activation(lse[:], sumexp[:], AF.Ln)
                lsm = ep.tile([128, nbt, OUT], f32)
                nc.vector.tensor_tensor(
                    lsm[:], shifted[:],
                    lse[:][:, :, None].broadcast_to([128, nbt, OUT]),
                    op=ALU.subtract)
                nc.sync.dma_start(
                    out_d.rearrange("(t p) o -> p t o", p=128), lsm[:])

    nc.compile()
    return nc


_NC_CACHE = {}


def _get_nc():
    if "nc" not in _NC_CACHE:
        _NC_CACHE["nc"] = build_nc()
    return _NC_CACHE["nc"]


def kernel(x, W1, gamma, beta, W2):
    x = np.ascontiguousarray(np.asarray(x), dtype=np.float32)
    W1 = np.ascontiguousarray(np.asarray(W1), dtype=np.float32)
    gamma = np.ascontiguousarray(np.asarray(gamma), dtype=np.float32)
    beta = np.ascontiguousarray(np.asarray(beta), dtype=np.float32)
    W2 = np.ascontiguousarray(np.asarray(W2), dtype=np.float32)

    nc = _get_nc()
    b_sh = B // N_CORES
    in_maps = [
        {
            "x": x[c * b_sh:(c + 1) * b_sh],
            "W1": W1,
            "gamma": gamma,
            "beta": beta,
            "W2": W2,
        }
        for c in range(N_CORES)
    ]
    res = bass_utils.run_bass_kernel_spmd(
        nc, in_maps, core_ids=list(range(N_CORES)))
    return np.concatenate(
        [res.results[c]["out"] for c in range(N_CORES)], axis=0)
